# revision 23
# baseline (speedup 1.0000x reference)
"""DiT forward kernel for 8 Trainium2 NeuronCores — fp8 DoubleRow version.

Sharding: data-parallel over batch B=8 (one batch element per core).

Key design vs the bf16 baseline (2.17ms):
- All conditioning (t/dt embeds, label emb, adaLN modulation) is computed on
  the host; the per-(core,layer) LayerNorm affine (gamma=1+scale, beta=shift)
  is folded into the qkv/mlp1 weights+biases, and the gates g_a/g_m are folded
  into the o-proj/mlp2 weights+biases. The device only ever computes plain
  LN (x-mean)*rstd.
- All projection matmuls (qkv, o, mlp1, mlp2) and attention (scores, AV, Z)
  run in fp8(e4m3) DoubleRow mode: 2x contraction per pass at 157 TF/s.
  Weights are pre-scaled by 64 on the host (fp8 dynamic range) and the 1/64
  is folded into the psum-evacuation ops / activation scales.
- Biases are injected through the matmul itself: activations carry a pad
  k-slab whose partition-0 row is ones, and the weight pad slab's row 0
  holds bias*64.
- rstd = exp(-0.5*ln(var+eps)) on the scalar engine: Ln and Exp share one
  activation table set, so the LN chain forces no table reloads between the
  attention exps.
- Z (softmax denominator) comes from a DR ones-matmul replicated over 64
  output rows, which doubles as the partition-broadcast needed to normalize.
"""

import math
import os
import sys
import types

sys.path.insert(0, "/opt/trn_rl_repo")

import numpy as np
import ml_dtypes

import concourse.bass as bass
import concourse.tile as tile
from concourse import bacc, mybir
from concourse.bass_utils import run_bass_kernel_spmd

F32 = mybir.dt.float32
BF16 = mybir.dt.bfloat16
FP8 = mybir.dt.float8e4
AF = mybir.ActivationFunctionType
OP = mybir.AluOpType
DR = mybir.MatmulPerfMode.DoubleRow
E4M3 = ml_dtypes.float8_e4m3

B = 8
CIN = 4
IMG = 64
P = 2
D = 384
NH = 6
L = 12
MLP = 4 * D
NCLS = 1000
FREQ = 256
COUT = 4
HP = IMG // P
N = HP * HP  # 1024 tokens
HD = D // NH  # 64
KT = D // 128  # 3 k-tiles of the model dim
NHALF = 2  # column halves of 512 tokens
WS = 64.0  # fp8 weight pre-scale
WSI = 1.0 / WS

# qdr/kdr layout: head h -> partition range 32*(h%3) (matmul base partition
# must be 0/32/64), slabs (2*(h//3), +1) = the two 32-dim halves of the head.
# psum block b == slab b. perm[b][p] = model q/k dim stored at (block b, col p)
def _qk_perm():
    perm = np.zeros((4, 128), np.int64)
    used = np.zeros((4, 128), bool)
    for h in range(NH):
        pr = h % 3
        s0 = 2 * (h // 3)
        for q in range(2):
            b = s0 + q
            for j in range(32):
                perm[b, 32 * pr + j] = 64 * h + 32 * q + j
                used[b, 32 * pr + j] = True
    return perm, used


QK_PERM, QK_USED = _qk_perm()


def _register_profile_hook():
    """The stub antenv lacks axon_hooks; register the NTFF hook ourselves."""
    if "antenv.axon_hooks" in sys.modules:
        return
    try:
        import antenv
        from trn_agent_boot.trn_boot import _ntff_profile_via_ctypes

        mod = types.ModuleType("antenv.axon_hooks")
        mod._hook = _ntff_profile_via_ctypes("/opt/axon/libaxon_pjrt.so")
        mod.get_axon_ntff_profile_hook = lambda: mod._hook
        mod.set_axon_ntff_profile_hook = lambda h: setattr(mod, "_hook", h)
        sys.modules["antenv.axon_hooks"] = mod
        antenv.axon_hooks = mod
    except Exception:
        pass


def _patched_act_tables():
    """Return an act-table view where Ln/Exp only co-reside in the shared
    natural_log_exp_and_others set, so the table-load chooser emits no
    reload between the LN rstd chain and the attention exps."""
    import concourse.bacc as bacc_mod

    orig = bacc_mod.get_activation_tables

    def wrapper(arch):
        t = dict(orig(arch))
        keep = t.get("natural_log_exp_and_others")
        if keep:
            for name, funcs in t.items():
                if name != "natural_log_exp_and_others" and (
                    AF.Exp in funcs or AF.Ln in funcs
                ):
                    t[name] = funcs - {AF.Exp, AF.Ln}
        return t

    return bacc_mod, orig, wrapper


def build_module(n_layers=L):
    bacc_mod, orig_tables, wrapper = _patched_act_tables()
    bacc_mod.get_activation_tables = wrapper
    try:
        return _build_module_inner(n_layers)
    finally:
        bacc_mod.get_activation_tables = orig_tables


def _build_module_inner(n_layers=L):
    nc = bacc.Bacc(None, target_bir_lowering=False)

    def din(name, shape, dtype=F32):
        return nc.declare_dram_parameter(name, list(shape), dtype, isOutput=False)

    H = {}
    # per-core activations
    H["xpT_hi"] = din("xpT_hi", [16, N], BF16)
    H["xpT_lo"] = din("xpT_lo", [16, N], BF16)
    # shared
    H["posT"] = din("posT", [128, KT, N])
    H["pw_hi"] = din("pw_hi", [16, D], BF16)
    H["pw_lo"] = din("pw_lo", [16, D], BF16)
    H["patch_b"] = din("patch_b", [128, KT])
    # per-core folded fp8 weights
    H["qw8"] = din("qw8", [L, 128, 4, 512], FP8)
    H["kw8"] = din("kw8", [L, 128, 4, 512], FP8)
    H["vw8"] = din("vw8", [L, 128, 4, D], FP8)
    H["ow8"] = din("ow8", [L, 128, 4, D], FP8)
    H["mw18"] = din("mw18", [L, 128, 4, MLP], FP8)
    H["mw28"] = din("mw28", [L, 128, 14, D], FP8)
    # per-core folded final projection (bf16 hi/lo)
    H["fpw_hi"] = din("fpw_hi", [128, KT, 16], BF16)
    H["fpw_lo"] = din("fpw_lo", [128, KT, 16], BF16)
    H["fpb"] = din("fpb", [16, 1])

    H["outT"] = nc.declare_dram_parameter("outT", [16, N], F32, isOutput=True)

    with tile.TileContext(nc) as tc:
        _emit(nc, tc, H, n_layers)
    nc.compile()
    return nc


def _emit(nc, tc, H, n_layers):
    import contextlib

    ctx = contextlib.ExitStack()
    with ctx:
        const = ctx.enter_context(tc.tile_pool(name="const", bufs=1))
        persist = ctx.enter_context(tc.tile_pool(name="persist", bufs=1))
        wpool = ctx.enter_context(tc.tile_pool(name="wpool", bufs=2))
        lnp = ctx.enter_context(tc.tile_pool(name="lnp", bufs=2))
        stat = ctx.enter_context(tc.tile_pool(name="stat", bufs=2))
        expp = ctx.enter_context(tc.tile_pool(name="expp", bufs=2))
        misc = ctx.enter_context(tc.tile_pool(name="misc", bufs=2))
        ps = ctx.enter_context(tc.tile_pool(name="ps", bufs=1, space="PSUM"))

        def ps_s():  # scores / patch psum [128, 1024]; 3-deep so the
            # score stream runs ahead of the exp consumer
            return ps.tile([128, N], F32, tag="s", bufs=3, name="ps_s")

        def ps_avh():  # per-half AV / Z psum [128, 512]
            return ps.tile([128, 512], F32, tag="avh", bufs=2, name="ps_avh")

        _mm_ctr = [0]

        def ps_mm():  # projection-block psum [128, 512]; rotates s/avh
            _mm_ctr[0] += 1
            if _mm_ctr[0] % 2:
                return ps.tile([128, 512], F32, tag="avh", bufs=2, name="ps_mm")
            return ps.tile([128, 512], F32, tag="s", bufs=3, name="ps_mm")

        def ps_mm_big():  # [128, 1024] block-pair psum
            return ps.tile([128, N], F32, tag="s", bufs=3, name="ps_mmb")

        # ---------------- constants ----------------
        ones_bf = const.tile([128, 128], BF16)
        nc.vector.memset(ones_bf, 1.0)
        # Z lhsT: A-variant cols 0:64 ones (out rows 0-63), B-variant 64:128
        ones8z = const.tile([128, 2, 256], FP8)
        nc.vector.memset(ones8z, 0.0)
        nc.vector.memset(ones8z[:, :, 0:64], 1.0)
        nc.vector.memset(ones8z[:, :, 192:256], 1.0)
        ones8dr = const.tile([128, 2, 128], FP8)
        nc.vector.memset(ones8dr, 1.0)
        eps_col = const.tile([128, 1], F32)
        nc.vector.memset(eps_col, 1e-6)

        # ---------------- persistent activations ----------------
        hT = persist.tile([128, KT, N], F32)
        xm8 = persist.tile([128, 4, N], FP8)
        xm28 = persist.tile([128, 4, N], FP8)
        qdr = persist.tile([128, 4, N], FP8)
        kdr = persist.tile([128, 4, N], FP8)
        # v, pair-padded: per pair hp, A-lhsT = cols [vA|0], B-lhsT = [0|vB]
        v8p = persist.tile([128, 8, 2 * D], FP8)
        nc.vector.memset(v8p, 0.0)
        oT8 = persist.tile([128, 4, N], FP8)
        g8 = persist.tile([128, 14, N], FP8)
        h8 = persist.tile([128, 4, N], FP8)
        sq8 = persist.tile([128, 4, N], FP8)
        nc.vector.memset(h8[:, 3, :], 0.0)
        nc.vector.memset(sq8[:, 3, :], 0.0)

        # pad slabs: row0 = 1, rest 0 (bias injection rows)
        for t in (xm8, xm28, oT8):
            nc.vector.memset(t[:, 3, :], 0.0)
            nc.vector.memset(t[0:1, 3, :], 1.0)
        nc.vector.memset(g8[:, 12, :], 0.0)
        nc.vector.memset(g8[0:1, 12, :], 1.0)
        nc.vector.memset(g8[:, 13, :], 0.0)

        # =========================================================
        # Patch embed (bf16 hi/lo, fp32-quality): hT = pwT @ xpT + b + pos
        # =========================================================
        xph = const.tile([16, N], BF16)
        xpl = const.tile([16, N], BF16)
        pwh = const.tile([16, D], BF16)
        pwl = const.tile([16, D], BF16)
        patch_b = const.tile([128, KT], F32)
        nc.sync.dma_start(out=xph, in_=H["xpT_hi"].ap())
        nc.sync.dma_start(out=xpl, in_=H["xpT_lo"].ap())
        nc.sync.dma_start(out=pwh, in_=H["pw_hi"].ap())
        nc.sync.dma_start(out=pwl, in_=H["pw_lo"].ap())
        nc.sync.dma_start(out=patch_b, in_=H["patch_b"].ap())

        for mt in range(KT):
            pp = ps_s()
            for half in range(NHALF):
                sl = slice(half * 512, half * 512 + 512)
                for i, (w, x) in enumerate(((pwh, xph), (pwh, xpl), (pwl, xph))):
                    nc.tensor.matmul(
                        pp[:, sl], w[:, mt * 128 : mt * 128 + 128], x[:, sl],
                        start=(i == 0), stop=(i == 2),
                    )
            for half in range(NHALF):
                sl = slice(half * 512, half * 512 + 512)
                post = misc.tile([128, 512], F32, tag="res", name="post")
                nc.sync.dma_start(out=post, in_=H["posT"].ap()[:, mt, sl])
                tmp = misc.tile([128, 512], F32, tag="res", name="tmp")
                nc.vector.tensor_scalar_add(
                    out=tmp, in0=pp[:, sl], scalar1=patch_b[:, mt : mt + 1]
                )
                nc.vector.tensor_tensor(
                    out=hT[:, mt, sl], in0=tmp, in1=post, op=OP.add
                )

        # =========================================================
        # LayerNorm -> fp8 slabs (no affine; folded into weights)
        # =========================================================
        def ln_stats(half, mstat):
            """Stats matmuls + mean/var/rstd for one 512-col half.

            mstat: dict receiving mean/rstd tiles keyed by half."""
            sl = slice(half * 512, half * 512 + 512)
            for k in range(KT):
                nc.vector.tensor_copy(h8[:, k, sl], hT[:, k, sl])
                nc.vector.tensor_tensor(
                    out=sq8[:, k, sl], in0=h8[:, k, sl], in1=h8[:, k, sl],
                    op=OP.mult,
                )
            pM = ps_mm()
            for t in range(2):
                nc.tensor.matmul(
                    pM, ones8dr, h8[:, 2 * t : 2 * t + 2, sl],
                    start=(t == 0), stop=(t == 1), perf_mode=DR,
                )
            pQ = ps_mm()
            for t in range(2):
                nc.tensor.matmul(
                    pQ, ones8dr, sq8[:, 2 * t : 2 * t + 2, sl],
                    start=(t == 0), stop=(t == 1), perf_mode=DR,
                )
            mean = stat.tile([128, 512], F32, tag="mean", name="mean")
            nc.vector.tensor_scalar(
                out=mean, in0=pM, scalar1=1.0 / D, scalar2=0.0,
                op0=OP.mult, op1=OP.add,
            )
            # d_ only needs the mean: run it on Pool in parallel with the
            # msq/var/ln/exp rstd chain below.
            ds = []
            for k in range(KT):
                d_ = stat.tile([128, 512], F32, tag="d_", bufs=3, name="d_")
                nc.gpsimd.tensor_tensor(
                    out=d_, in0=hT[:, k, sl], in1=mean, op=OP.subtract
                )
                ds.append(d_)
            msq = stat.tile([128, 512], F32, tag="msq", name="msq")
            nc.vector.tensor_tensor(out=msq, in0=mean, in1=mean, op=OP.mult)
            var = stat.tile([128, 512], F32, tag="var", name="var")
            nc.vector.scalar_tensor_tensor(
                out=var, in0=pQ, scalar=1.0 / D, in1=msq,
                op0=OP.mult, op1=OP.subtract,
            )
            lnv = stat.tile([128, 512], F32, tag="lnv", name="lnv")
            nc.scalar.activation(lnv, var, AF.Ln, bias=eps_col)
            rstd = stat.tile([128, 512], F32, tag="rstd", name="rstd")
            nc.scalar.activation(rstd, lnv, AF.Exp, scale=-0.5)
            mstat[half] = (ds, rstd)

        def ln_chain(dst8, half, mstat):
            """dst8[:, k, half] = (hT - mean)*rstd, fp8."""
            sl = slice(half * 512, half * 512 + 512)
            ds, rstd = mstat[half]
            for k in range(KT):
                nc.gpsimd.tensor_tensor(
                    out=dst8[:, k, sl], in0=ds[k], in1=rstd, op=OP.mult
                )

        # =========================================================
        # fp8 DR projection: out_blocks x halves, contraction via slab pairs
        # =========================================================
        def proj(w_t, src8, nblocks, npairs, evac):
            """psum[b, half] = sum_t w[:, 2t:2t+2, 128b:+128]^T @ src8 pairs."""
            for b_ in range(nblocks):
                pr = [None, None]
                for half in range(NHALF):
                    sl = slice(half * 512, half * 512 + 512)
                    pr[half] = ps_mm()
                    for t in range(npairs):
                        nc.tensor.matmul(
                            pr[half],
                            w_t[:, 2 * t : 2 * t + 2, 128 * b_ : 128 * b_ + 128],
                            src8[:, 2 * t : 2 * t + 2, sl],
                            start=(t == 0), stop=(t == npairs - 1),
                            perf_mode=DR,
                        )
                for half in range(NHALF):
                    evac(pr[half], b_, half)

        # =========================================================
        # Transformer layers
        # =========================================================
        for l in range(n_layers):
            qw = wpool.tile([128, 4, 512], FP8, tag="qw", name=f"qw{l}")
            kw = wpool.tile([128, 4, 512], FP8, tag="kw", name=f"kw{l}")
            vw = wpool.tile([128, 4, D], FP8, tag="vw", name=f"vw{l}")
            ow = wpool.tile([128, 4, D], FP8, tag="ow", name=f"ow{l}")
            mw1 = wpool.tile([128, 4, MLP], FP8, tag="mw1", name=f"mw1{l}")
            mw2 = wpool.tile([128, 14, D], FP8, tag="mw2", name=f"mw2{l}")
            nc.sync.dma_start(out=qw, in_=H["qw8"].ap()[l])
            nc.sync.dma_start(out=kw, in_=H["kw8"].ap()[l])
            nc.sync.dma_start(out=vw, in_=H["vw8"].ap()[l])
            nc.sync.dma_start(out=ow, in_=H["ow8"].ap()[l])
            nc.sync.dma_start(out=mw1, in_=H["mw18"].ap()[l])
            nc.sync.dma_start(out=mw2, in_=H["mw28"].ap()[l])

            # ---- LN1 (h0 was emitted under the previous layer's mlp2) ----
            if l == 0:
                ms1 = {}
                ln_stats(0, ms1)
                ln_chain(xm8, 0, ms1)
            ms1b = {}
            ln_stats(1, ms1b)
            ln_chain(xm8, 1, ms1b)

            # ---- q/k into qdr layout; v into v8p (half-outer) ----
            for half in range(NHALF):
                sl = slice(half * 512, half * 512 + 512)
                for w_t, dst in ((qw, qdr), (kw, kdr)):
                    for b_ in range(4):
                        pr_ = ps_mm()
                        for t in range(2):
                            nc.tensor.matmul(
                                pr_,
                                w_t[:, 2 * t : 2 * t + 2,
                                    128 * b_ : 128 * b_ + 128],
                                xm8[:, 2 * t : 2 * t + 2, sl],
                                start=(t == 0), stop=(t == 1),
                                perf_mode=DR,
                            )
                        nc.vector.tensor_scalar(
                            out=dst[:, b_, sl], in0=pr_, scalar1=WSI,
                            scalar2=0.0, op0=OP.mult, op1=OP.add,
                        )

            def emit_vproj():
                for jt in range(8):
                    jsl = slice(jt * 128, jt * 128 + 128)
                    pv = ps.tile([128, 512], F32, tag="avh", bufs=2, name="ps_v")
                    for t in range(2):
                        nc.tensor.matmul(
                            pv[:, :D],
                            xm8[:, 2 * t : 2 * t + 2, jsl],
                            vw[:, 2 * t : 2 * t + 2, :],
                            start=(t == 0), stop=(t == 1),
                            perf_mode=DR,
                        )
                    vdst = bass.AP(
                        tensor=v8p.tensor,
                        offset=v8p.offset + jt * (2 * D),
                        ap=[list(v8p.ap[0]), [256, KT], [192, 2], [1, 64]],
                    )
                    pv_v = bass.AP(
                        tensor=pv.tensor, offset=pv.offset,
                        ap=[list(pv.ap[0]), [128, KT], [64, 2], [1, 64]],
                    )
                    nc.vector.tensor_scalar(
                        out=vdst, in0=pv_v, scalar1=WSI, scalar2=0.0,
                        op0=OP.mult, op1=OP.add,
                    )

            # ---- attention: pair-pipelined (scores/exp of pair p+1 are
            # emitted before AV/Z of pair p so the ACT exp stream never
            # stalls on the PE's AV/Z tail) ----
            def score_step(hp, par, jt, ex):
                h = 2 * hp + par
                prr = h % 3
                s0 = 2 * (h // 3)
                po = slice(32 * prr, 32 * prr + 32)
                jsl = slice(jt * 128, jt * 128 + 128)
                sp = ps_s()
                for half in range(NHALF):
                    sl = slice(half * 512, half * 512 + 512)
                    nc.tensor.matmul(
                        sp[:, sl],
                        kdr[po, s0 : s0 + 2, jsl],
                        qdr[po, s0 : s0 + 2, sl],
                        start=True, stop=True,
                        perf_mode=DR,
                    )
                nc.scalar.activation(ex[:, jt, :], sp, AF.Exp, scale=1.0 / HD)

            def make_avz(hp, exs):
                state = {}
                steps = []
                for half in range(NHALF):
                    sl = slice(half * 512, half * 512 + 512)
                    for par in range(2):
                        for jp in range(4):
                            def _step(half=half, sl=sl, par=par, jp=jp):
                                if par == 0 and jp == 0:
                                    state["pav"] = ps_avh()
                                    state["pz"] = ps_avh()
                                vsl = slice(256 * hp + 128 * par,
                                            256 * hp + 128 * par + 128)
                                osl = slice(128 * par, 128 * par + 128)
                                st = par == 0 and jp == 0
                                sp_ = par == 1 and jp == 3
                                nc.tensor.matmul(
                                    state["pav"],
                                    v8p[:, 2 * jp : 2 * jp + 2, vsl],
                                    exs[par][:, 2 * jp : 2 * jp + 2, sl],
                                    start=st, stop=sp_, perf_mode=DR,
                                )
                                nc.tensor.matmul(
                                    state["pz"],
                                    ones8z[:, :, osl],
                                    exs[par][:, 2 * jp : 2 * jp + 2, sl],
                                    start=st, stop=sp_, perf_mode=DR,
                                )
                                if sp_:
                                    zrec = expp.tile(
                                        [128, 512], F32, tag="zrec", name="zrec"
                                    )
                                    with nc.allow_low_precision(
                                        reason="1/Z at 18 bits"
                                    ):
                                        nc.vector.reciprocal_approx_fast(
                                            out=zrec, in_=state["pz"]
                                        )
                                    nc.vector.tensor_tensor(
                                        out=oT8[:, hp, sl], in0=state["pav"],
                                        in1=zrec, op=OP.mult,
                                    )
                            steps.append(_step)

                def norm():
                    pass

                return steps, norm

            def vproj_step(jt):
                jsl = slice(jt * 128, jt * 128 + 128)
                pv = ps.tile([128, 512], F32, tag="avh", bufs=2, name="ps_v")
                for t in range(2):
                    nc.tensor.matmul(
                        pv[:, :D],
                        xm8[:, 2 * t : 2 * t + 2, jsl],
                        vw[:, 2 * t : 2 * t + 2, :],
                        start=(t == 0), stop=(t == 1),
                        perf_mode=DR,
                    )
                vdst = bass.AP(
                    tensor=v8p.tensor,
                    offset=v8p.offset + jt * (2 * D),
                    ap=[list(v8p.ap[0]), [256, KT], [192, 2], [1, 64]],
                )
                pv_v = bass.AP(
                    tensor=pv.tensor, offset=pv.offset,
                    ap=[list(pv.ap[0]), [128, KT], [64, 2], [1, 64]],
                )
                nc.vector.tensor_scalar(
                    out=vdst, in0=pv_v, scalar1=WSI, scalar2=0.0,
                    op0=OP.mult, op1=OP.add,
                )

            # scores/exp of pair p interleaved, step-by-step, with AV/Z of
            # pair p-1 (and with v-proj during pair 0) so the PE never
            # drains while the ACT exp stream runs.
            avz_steps = avz_norm = None
            for hp in range(NH // 2):
                exs = [
                    expp.tile([128, 8, N], FP8, tag="ex", bufs=4, name=f"ex{2*hp+par}")
                    for par in range(2)
                ]
                for i in range(16):
                    par, jt = divmod(i, 8)
                    score_step(hp, par, jt, exs[par])
                    if avz_steps is not None:
                        avz_steps[i]()
                    elif i % 2 == 1:
                        vproj_step(i // 2)
                if avz_norm is not None:
                    avz_norm()
                avz_steps, avz_norm = make_avz(hp, exs)
            for st in avz_steps:
                st()
            avz_norm()

            # ---- o-proj + residual (half-outer) ----
            def proj_half(w_t, src8, nblocks, npairs, half, evac):
                sl = slice(half * 512, half * 512 + 512)
                for b_ in range(nblocks):
                    pr_ = ps_mm()
                    for t in range(npairs):
                        nc.tensor.matmul(
                            pr_,
                            w_t[:, 2 * t : 2 * t + 2,
                                128 * b_ : 128 * b_ + 128],
                            src8[:, 2 * t : 2 * t + 2, sl],
                            start=(t == 0), stop=(t == npairs - 1),
                            perf_mode=DR,
                        )
                    evac(pr_, b_, sl)

            def evac_resid(pr_, b_, sl):
                nc.vector.scalar_tensor_tensor(
                    out=hT[:, b_, sl], in0=pr_, scalar=WSI,
                    in1=hT[:, b_, sl], op0=OP.mult, op1=OP.add,
                )

            proj_half(ow, oT8, 3, 2, 0, evac_resid)
            proj_half(ow, oT8, 3, 2, 1, evac_resid)
            ms2 = {}
            ln_stats(0, ms2)
            ln_chain(xm28, 0, ms2)
            ms2b = {}
            ln_stats(1, ms2b)
            ln_chain(xm28, 1, ms2b)

            # mlp1: pair two 128-row blocks per [128, 1024] psum; one gelu
            # per pair writes both g8 slabs (strided out AP)
            for half in range(NHALF):
                sl = slice(half * 512, half * 512 + 512)
                for bp in range(MLP // 256):
                    pg = ps_mm_big()
                    for sub in range(2):
                        b_ = 2 * bp + sub
                        psl = slice(sub * 512, sub * 512 + 512)
                        for t in range(2):
                            nc.tensor.matmul(
                                pg[:, psl],
                                mw1[:, 2 * t : 2 * t + 2,
                                    128 * b_ : 128 * b_ + 128],
                                xm28[:, 2 * t : 2 * t + 2, sl],
                                start=(t == 0), stop=(t == 1),
                                perf_mode=DR,
                            )
                    gdst = bass.AP(
                        tensor=g8.tensor,
                        offset=g8.offset + 2 * bp * N + half * 512,
                        ap=[list(g8.ap[0]), [N, 2], [1, 512]],
                    )
                    nc.scalar.activation(gdst, pg, AF.Gelu, scale=WSI)

            # mlp2 (half-outer so next layer's LN1(h0) can start early)
            proj_half(mw2, g8, 3, 7, 0, evac_resid)
            if l + 1 < n_layers:
                # next layer's LN1 half0 hides under mlp2 half1
                ms1n = {}
                ln_stats(0, ms1n)
                proj_half(mw2, g8, 3, 7, 1, evac_resid)
                ln_chain(xm8, 0, ms1n)
            else:
                proj_half(mw2, g8, 3, 7, 1, evac_resid)

        # =========================================================
        # Final layer: plain LN (affine folded into fpw) in bf16 hi/lo
        # =========================================================
        fpwh = const.tile([128, KT, 16], BF16)
        fpwl = const.tile([128, KT, 16], BF16)
        fpb = const.tile([16, 1], F32)
        nc.sync.dma_start(out=fpwh, in_=H["fpw_hi"].ap())
        nc.sync.dma_start(out=fpwl, in_=H["fpw_lo"].ap())
        nc.sync.dma_start(out=fpb, in_=H["fpb"].ap())

        xmF = lnp.tile([128, KT, N], BF16, tag="hTb", name="xmF")
        xmFlo = lnp.tile([128, KT, N], BF16, tag="sq", name="xmFlo")
        hTb = lnp.tile([128, KT, N], BF16, tag="hTb", name="hTbF")
        sq = lnp.tile([128, KT, N], BF16, tag="sq", name="sqF")
        for k in range(KT):
            nc.vector.tensor_copy(hTb[:, k, :], hT[:, k, :])
            nc.vector.tensor_tensor(
                out=sq[:, k, :], in0=hTb[:, k, :], in1=hTb[:, k, :], op=OP.mult
            )
        for half in range(NHALF):
            sl = slice(half * 512, half * 512 + 512)
            pM = ps_mm()
            for k in range(KT):
                nc.tensor.matmul(
                    pM, ones_bf, hTb[:, k, sl],
                    start=(k == 0), stop=(k == KT - 1),
                )
            pQ = ps_mm()
            for k in range(KT):
                nc.tensor.matmul(
                    pQ, ones_bf, sq[:, k, sl],
                    start=(k == 0), stop=(k == KT - 1),
                )
            mean = stat.tile([128, 512], F32, tag="mean", name="meanF")
            nc.vector.tensor_scalar(
                out=mean, in0=pM, scalar1=1.0 / D, scalar2=0.0,
                op0=OP.mult, op1=OP.add,
            )
            msq = stat.tile([128, 512], F32, tag="msq", name="msqF")
            nc.vector.tensor_tensor(out=msq, in0=mean, in1=mean, op=OP.mult)
            var = stat.tile([128, 512], F32, tag="var", name="varF")
            nc.vector.scalar_tensor_tensor(
                out=var, in0=pQ, scalar=1.0 / D, in1=msq,
                op0=OP.mult, op1=OP.subtract,
            )
            lnv = stat.tile([128, 512], F32, tag="lnv", name="lnvF")
            nc.scalar.activation(lnv, var, AF.Ln, bias=eps_col)
            rstd = stat.tile([128, 512], F32, tag="rstd", name="rstdF")
            nc.scalar.activation(rstd, lnv, AF.Exp, scale=-0.5)
            for k in range(KT):
                d_ = stat.tile([128, 512], F32, tag="d_", bufs=3, name="dF")
                nc.vector.scalar_tensor_tensor(
                    out=d_, in0=mean, scalar=-1.0, in1=hT[:, k, sl],
                    op0=OP.mult, op1=OP.add,
                )
                ef = stat.tile([128, 512], F32, tag="ef", name="ef")
                nc.vector.tensor_tensor(out=ef, in0=d_, in1=rstd, op=OP.mult)
                nc.vector.tensor_copy(xmF[:, k, sl], ef)
                nc.vector.scalar_tensor_tensor(
                    out=xmFlo[:, k, sl], in0=xmF[:, k, sl], scalar=-1.0,
                    in1=ef, op0=OP.mult, op1=OP.add,
                )

        out_sb = misc.tile([16, N], F32, tag="outsb", bufs=1, name="out_sb")
        for half in range(NHALF):
            sl = slice(half * 512, half * 512 + 512)
            pf = ps_mm()
            mms = []
            for k in range(KT):
                mms.append((fpwh[:, k, :], xmF[:, k, sl]))
                mms.append((fpwl[:, k, :], xmF[:, k, sl]))
                mms.append((fpwh[:, k, :], xmFlo[:, k, sl]))
            for i, (wv, xv) in enumerate(mms):
                nc.tensor.matmul(
                    pf[:16, :], wv, xv,
                    start=(i == 0), stop=(i == len(mms) - 1),
                )
            nc.vector.tensor_scalar_add(
                out=out_sb[:, sl], in0=pf[:16, :], scalar1=fpb
            )
        nc.sync.dma_start(out=H["outT"].ap(), in_=out_sb)


# =================================================================
# Host side
# =================================================================
_BUILD_CACHE = {}


def _get_module(n_layers=L):
    if n_layers not in _BUILD_CACHE:
        _register_profile_hook()
        _BUILD_CACHE[n_layers] = build_module(n_layers)
    return _BUILD_CACHE[n_layers]


def _silu(x):
    return x / (1.0 + np.exp(-x))


def _t_embed_np(t, w1, b1, w2, b2):
    half = FREQ // 2
    freqs = np.exp(-math.log(10000.0) * np.arange(half, dtype=np.float64) / half)
    a = t.astype(np.float64)[:, None] * freqs[None]
    e = np.concatenate([np.cos(a), np.sin(a)], -1)
    return _silu(e @ w1 + b1) @ w2 + b2


def _fp8(x):
    return np.clip(np.asarray(x, np.float32), -440.0, 440.0).astype(E4M3)


def _shuf_w(w):
    """[Din, Dout] -> [128, Din//128, Dout], partition-contiguous."""
    din = w.shape[0]
    return np.ascontiguousarray(w.reshape(din // 128, 128, -1).transpose(1, 0, 2))


def _dr_w(w_f, bias, dout_pad=None):
    """[D, Dout] + bias[Dout] -> [128, 4, Dout] fp8 with pad slab bias row."""
    dout = w_f.shape[1] if dout_pad is None else dout_pad
    out = np.zeros((128, 4, dout), np.float32)
    s = _shuf_w(w_f * WS)  # [128, 3, Dout]
    out[:, 0:3, : w_f.shape[1]] = s
    out[0, 3, : w_f.shape[1]] = bias * WS
    return _fp8(out)


def prepare_inputs(inputs, n_layers=L):
    ii = {k: np.asarray(v) for k, v in inputs.items()}
    x = ii["x"].astype(np.float32)
    t = ii["t"].astype(np.float64)
    dt = ii["dt"].astype(np.float64)
    y = ii["y"].astype(np.int64)
    pos = ii["pos"].astype(np.float32)

    # ---- conditioning on host ----
    c = (
        _t_embed_np(t, ii["t1_w1"].astype(np.float64), ii["t1_b1"].astype(np.float64),
                    ii["t1_w2"].astype(np.float64), ii["t1_b2"].astype(np.float64))
        + _t_embed_np(dt, ii["t2_w1"].astype(np.float64), ii["t2_b1"].astype(np.float64),
                      ii["t2_w2"].astype(np.float64), ii["t2_b2"].astype(np.float64))
        + ii["label_emb"].astype(np.float64)[y]
    )  # [B, D]
    ca = _silu(c)  # [B, D]

    shared = {}
    shared["posT"] = _shuf_w(np.ascontiguousarray(pos.T))
    pw = ii["patch_w"].astype(np.float32)
    pwh = pw.astype(ml_dtypes.bfloat16)
    shared["pw_hi"] = pwh
    shared["pw_lo"] = (pw - pwh.astype(np.float32)).astype(ml_dtypes.bfloat16)
    shared["patch_b"] = np.ascontiguousarray(
        ii["patch_b"].astype(np.float32).reshape(-1, 128).T
    )

    in_maps = []
    for b in range(B):
        m = dict(shared)
        xp = (
            x[b]
            .reshape(CIN, HP, P, HP, P)
            .transpose(1, 3, 0, 2, 4)
            .reshape(N, CIN * P * P)
        )
        xpT = np.ascontiguousarray(xp.T)
        xph = xpT.astype(ml_dtypes.bfloat16)
        m["xpT_hi"] = xph
        m["xpT_lo"] = (xpT - xph.astype(np.float32)).astype(ml_dtypes.bfloat16)

        qw8 = np.zeros((L, 128, 4, 512), E4M3)
        kw8 = np.zeros((L, 128, 4, 512), E4M3)
        vw8 = np.zeros((L, 128, 4, D), E4M3)
        ow8 = np.zeros((L, 128, 4, D), E4M3)
        mw18 = np.zeros((L, 128, 4, MLP), E4M3)
        mw28 = np.zeros((L, 128, 14, D), E4M3)

        for l in range(n_layers):
            mod = ca[b] @ ii["adaln_w"][l].astype(np.float64) + ii["adaln_b"][
                l
            ].astype(np.float64)
            sh_a, sc_a, g_a, sh_m, sc_m, g_m = np.split(mod, 6)
            ga = (1.0 + sc_a).astype(np.float32)
            ba = sh_a.astype(np.float32)
            gm = (1.0 + sc_m).astype(np.float32)
            bm = sh_m.astype(np.float32)
            g_a = g_a.astype(np.float32)
            g_m = g_m.astype(np.float32)

            # q/k: fold LN affine; permute out cols into qdr block layout
            for nm, dst in (("q", qw8), ("k", kw8)):
                w_f = ga[:, None] * ii[f"{nm}_w"][l].astype(np.float32)
                b_f = ii[f"{nm}_b"][l].astype(np.float32) + ba @ ii[f"{nm}_w"][
                    l
                ].astype(np.float32)
                wp = np.zeros((D, 512), np.float32)
                bp = np.zeros((512,), np.float32)
                for blk in range(4):
                    cols = QK_PERM[blk][QK_USED[blk]]
                    idx = np.nonzero(QK_USED[blk])[0]
                    wp[:, 128 * blk + idx] = w_f[:, cols]
                    bp[128 * blk + idx] = b_f[cols]
                dst[l] = _dr_w(wp, bp)

            # v: fold LN affine; bias folded into o bias
            vw_f = ga[:, None] * ii["v_w"][l].astype(np.float32)
            vb_eff = ii["v_b"][l].astype(np.float32) + ba @ ii["v_w"][l].astype(
                np.float32
            )
            vw8[l] = _dr_w(vw_f, np.zeros(D, np.float32))

            # o: fold gate; bias absorbs v bias
            ow_f = ii["o_w"][l].astype(np.float32) * g_a[None, :]
            ob_f = g_a * (
                ii["o_b"][l].astype(np.float32)
                + vb_eff @ ii["o_w"][l].astype(np.float32)
            )
            ow8[l] = _dr_w(ow_f, ob_f)

            # mlp1: fold LN affine
            mw1_f = gm[:, None] * ii["m_w1"][l].astype(np.float32)
            mb1_f = ii["m_b1"][l].astype(np.float32) + bm @ ii["m_w1"][l].astype(
                np.float32
            )
            mw18[l] = _dr_w(mw1_f, mb1_f)

            # mlp2: fold gate; 14-slab layout with bias row in slab 12
            mw2_f = ii["m_w2"][l].astype(np.float32) * g_m[None, :]
            mb2_f = g_m * ii["m_b2"][l].astype(np.float32)
            w14 = np.zeros((128, 14, D), np.float32)
            w14[:, 0:12, :] = _shuf_w(mw2_f * WS)
            w14[0, 12, :] = mb2_f * WS
            mw28[l] = _fp8(w14)

        m["qw8"] = qw8
        m["kw8"] = kw8
        m["vw8"] = vw8
        m["ow8"] = ow8
        m["mw18"] = mw18
        m["mw28"] = mw28

        # final layer fold
        finm = _silu(c[b]) @ ii["fin_mw"].astype(np.float64) + ii["fin_mb"].astype(
            np.float64
        )
        sh_f, sc_f = np.split(finm, 2)
        gf = (1.0 + sc_f).astype(np.float32)
        bf = sh_f.astype(np.float32)
        fpw_f = gf[:, None] * ii["fin_pw"].astype(np.float32)
        fpb_f = ii["fin_pb"].astype(np.float32) + bf @ ii["fin_pw"].astype(
            np.float32
        )
        fpw_s = _shuf_w(fpw_f)
        fpwh = fpw_s.astype(ml_dtypes.bfloat16)
        m["fpw_hi"] = fpwh
        m["fpw_lo"] = (fpw_s - fpwh.astype(np.float32)).astype(ml_dtypes.bfloat16)
        m["fpb"] = fpb_f.reshape(16, 1)
        in_maps.append(m)
    return in_maps


def assemble_output(results):
    out = np.empty((B, COUT, IMG, IMG), np.float32)
    for b in range(B):
        tok = results[b]["outT"].T  # [N, 16]
        out[b] = (
            tok.reshape(HP, HP, P, P, COUT)
            .transpose(4, 0, 2, 1, 3)
            .reshape(COUT, IMG, IMG)
        )
    return out


def run(inputs, n_layers=L, trace=False, sim=False):
    nc = _get_module(n_layers)
    in_maps = prepare_inputs(inputs, n_layers)
    if sim:
        from concourse.bass_interp import CoreSim

        s = CoreSim(nc, trace=False)
        for k, v in in_maps[0].items():
            s.tensor(k)[:] = v
        s.simulate()
        results = [{"outT": np.array(s.tensor("outT"))} for _ in range(B)]
        return results, None
    res = run_bass_kernel_spmd(
        nc, in_maps, core_ids=list(range(B)), trace=trace
    )
    return res.results, res


def kernel(**inputs):
    results, _ = run(inputs, L, trace=False, sim=False)
    return assemble_output(results)


# revision 24
# speedup vs baseline: 1.0493x; 1.0493x over previous
"""DiT forward kernel for 8 Trainium2 NeuronCores — fp8 DoubleRow version.

Sharding: data-parallel over batch B=8 (one batch element per core).

Key design vs the bf16 baseline (2.17ms):
- All conditioning (t/dt embeds, label emb, adaLN modulation) is computed on
  the host; the per-(core,layer) LayerNorm affine (gamma=1+scale, beta=shift)
  is folded into the qkv/mlp1 weights+biases, and the gates g_a/g_m are folded
  into the o-proj/mlp2 weights+biases. The device only ever computes plain
  LN (x-mean)*rstd.
- All projection matmuls (qkv, o, mlp1, mlp2) and attention (scores, AV, Z)
  run in fp8(e4m3) DoubleRow mode: 2x contraction per pass at 157 TF/s.
  Weights are pre-scaled by 64 on the host (fp8 dynamic range) and the 1/64
  is folded into the psum-evacuation ops / activation scales.
- Biases are injected through the matmul itself: activations carry a pad
  k-slab whose partition-0 row is ones, and the weight pad slab's row 0
  holds bias*64.
- rstd = exp(-0.5*ln(var+eps)) on the scalar engine: Ln and Exp share one
  activation table set, so the LN chain forces no table reloads between the
  attention exps.
- Z (softmax denominator) comes from a DR ones-matmul replicated over 64
  output rows, which doubles as the partition-broadcast needed to normalize.
"""

import math
import os
import sys
import types

sys.path.insert(0, "/opt/trn_rl_repo")

import numpy as np
import ml_dtypes

import concourse.bass as bass
import concourse.tile as tile
from concourse import bacc, mybir
from concourse.bass_utils import run_bass_kernel_spmd

F32 = mybir.dt.float32
BF16 = mybir.dt.bfloat16
FP8 = mybir.dt.float8e4
AF = mybir.ActivationFunctionType
OP = mybir.AluOpType
DR = mybir.MatmulPerfMode.DoubleRow
E4M3 = ml_dtypes.float8_e4m3

B = 8
CIN = 4
IMG = 64
P = 2
D = 384
NH = 6
L = 12
MLP = 4 * D
NCLS = 1000
FREQ = 256
COUT = 4
HP = IMG // P
N = HP * HP  # 1024 tokens
HD = D // NH  # 64
KT = D // 128  # 3 k-tiles of the model dim
NHALF = 2  # column halves of 512 tokens
WS = 64.0  # fp8 weight pre-scale
WSI = 1.0 / WS

# qdr/kdr layout: head h -> partition range 32*(h%3) (matmul base partition
# must be 0/32/64), slabs (2*(h//3), +1) = the two 32-dim halves of the head.
# psum block b == slab b. perm[b][p] = model q/k dim stored at (block b, col p)
def _qk_perm():
    perm = np.zeros((4, 128), np.int64)
    used = np.zeros((4, 128), bool)
    for h in range(NH):
        pr = h % 3
        s0 = 2 * (h // 3)
        for q in range(2):
            b = s0 + q
            for j in range(32):
                perm[b, 32 * pr + j] = 64 * h + 32 * q + j
                used[b, 32 * pr + j] = True
    return perm, used


QK_PERM, QK_USED = _qk_perm()


def _register_profile_hook():
    """The stub antenv lacks axon_hooks; register the NTFF hook ourselves."""
    if "antenv.axon_hooks" in sys.modules:
        return
    try:
        import antenv
        from trn_agent_boot.trn_boot import _ntff_profile_via_ctypes

        mod = types.ModuleType("antenv.axon_hooks")
        mod._hook = _ntff_profile_via_ctypes("/opt/axon/libaxon_pjrt.so")
        mod.get_axon_ntff_profile_hook = lambda: mod._hook
        mod.set_axon_ntff_profile_hook = lambda h: setattr(mod, "_hook", h)
        sys.modules["antenv.axon_hooks"] = mod
        antenv.axon_hooks = mod
    except Exception:
        pass


def _patched_act_tables():
    """Return an act-table view where Ln/Exp only co-reside in the shared
    natural_log_exp_and_others set, so the table-load chooser emits no
    reload between the LN rstd chain and the attention exps."""
    import concourse.bacc as bacc_mod

    orig = bacc_mod.get_activation_tables

    def wrapper(arch):
        t = dict(orig(arch))
        keep = t.get("natural_log_exp_and_others")
        if keep:
            for name, funcs in t.items():
                if name != "natural_log_exp_and_others" and (
                    AF.Exp in funcs or AF.Ln in funcs
                ):
                    t[name] = funcs - {AF.Exp, AF.Ln}
        return t

    return bacc_mod, orig, wrapper


def build_module(n_layers=L):
    bacc_mod, orig_tables, wrapper = _patched_act_tables()
    bacc_mod.get_activation_tables = wrapper
    try:
        return _build_module_inner(n_layers)
    finally:
        bacc_mod.get_activation_tables = orig_tables


def _build_module_inner(n_layers=L):
    nc = bacc.Bacc(None, target_bir_lowering=False)

    def din(name, shape, dtype=F32):
        return nc.declare_dram_parameter(name, list(shape), dtype, isOutput=False)

    H = {}
    # per-core activations
    H["xpT_hi"] = din("xpT_hi", [16, N], BF16)
    H["xpT_lo"] = din("xpT_lo", [16, N], BF16)
    # shared
    H["posT"] = din("posT", [128, KT, N])
    H["pw_hi"] = din("pw_hi", [16, D], BF16)
    H["pw_lo"] = din("pw_lo", [16, D], BF16)
    H["patch_b"] = din("patch_b", [128, KT])
    # per-core folded fp8 weights
    H["qw8"] = din("qw8", [L, 128, 4, 512], FP8)
    H["kw8"] = din("kw8", [L, 128, 4, 512], FP8)
    H["vw8"] = din("vw8", [L, 128, 4, D], FP8)
    H["ow8"] = din("ow8", [L, 128, 4, D], FP8)
    H["mw18"] = din("mw18", [L, 128, 4, MLP], FP8)
    H["mw28"] = din("mw28", [L, 128, 14, D], FP8)
    # per-core folded final projection (bf16 hi/lo)
    H["fpw_hi"] = din("fpw_hi", [128, KT, 16], BF16)
    H["fpw_lo"] = din("fpw_lo", [128, KT, 16], BF16)
    H["fpb"] = din("fpb", [16, 1])

    H["outT"] = nc.declare_dram_parameter("outT", [16, N], F32, isOutput=True)

    with tile.TileContext(nc) as tc:
        _emit(nc, tc, H, n_layers)
    nc.compile()
    return nc


def _emit(nc, tc, H, n_layers):
    import contextlib

    ctx = contextlib.ExitStack()
    with ctx:
        const = ctx.enter_context(tc.tile_pool(name="const", bufs=1))
        persist = ctx.enter_context(tc.tile_pool(name="persist", bufs=1))
        wpool = ctx.enter_context(tc.tile_pool(name="wpool", bufs=2))
        lnp = ctx.enter_context(tc.tile_pool(name="lnp", bufs=2))
        stat = ctx.enter_context(tc.tile_pool(name="stat", bufs=2))
        expp = ctx.enter_context(tc.tile_pool(name="expp", bufs=2))
        misc = ctx.enter_context(tc.tile_pool(name="misc", bufs=2))
        ps = ctx.enter_context(tc.tile_pool(name="ps", bufs=1, space="PSUM"))

        def ps_s():  # scores / patch psum [128, 1024]; 3-deep so the
            # score stream runs ahead of the exp consumer
            return ps.tile([128, N], F32, tag="s", bufs=3, name="ps_s")

        def ps_avh():  # per-half AV / Z psum [128, 512]
            return ps.tile([128, 512], F32, tag="avh", bufs=2, name="ps_avh")

        _mm_ctr = [0]

        def ps_mm():  # projection-block psum [128, 512]; rotates s/avh
            _mm_ctr[0] += 1
            if _mm_ctr[0] % 2:
                return ps.tile([128, 512], F32, tag="avh", bufs=2, name="ps_mm")
            return ps.tile([128, 512], F32, tag="s", bufs=3, name="ps_mm")

        def ps_mm_big():  # [128, 1024] block-pair psum
            return ps.tile([128, N], F32, tag="s", bufs=3, name="ps_mmb")

        # ---------------- constants ----------------
        ones_bf = const.tile([128, 128], BF16)
        nc.vector.memset(ones_bf, 1.0)
        # Z lhsT: A-variant cols 0:64 ones (out rows 0-63), B-variant 64:128
        ones8z = const.tile([128, 2, 256], FP8)
        nc.vector.memset(ones8z, 0.0)
        nc.vector.memset(ones8z[:, :, 0:64], 1.0)
        nc.vector.memset(ones8z[:, :, 192:256], 1.0)
        ones8dr = const.tile([128, 2, 128], FP8)
        nc.vector.memset(ones8dr, 1.0)
        eps_col = const.tile([128, 1], F32)
        nc.vector.memset(eps_col, 1e-6)

        # ---------------- persistent activations ----------------
        hT = persist.tile([128, KT, N], F32)
        xm8 = persist.tile([128, 4, N], FP8)
        xm28 = persist.tile([128, 4, N], FP8)
        qdr = persist.tile([128, 4, N], FP8)
        kdr = persist.tile([128, 4, N], FP8)
        # v, pair-padded: per pair hp, A-lhsT = cols [vA|0], B-lhsT = [0|vB]
        v8p = persist.tile([128, 8, 2 * D], FP8)
        nc.vector.memset(v8p, 0.0)
        oT8 = persist.tile([128, 4, N], FP8)
        g8 = persist.tile([128, 14, N], FP8)

        # pad slabs: row0 = 1, rest 0 (bias injection rows)
        for t in (xm8, xm28, oT8):
            nc.vector.memset(t[:, 3, :], 0.0)
            nc.vector.memset(t[0:1, 3, :], 1.0)
        nc.vector.memset(g8[:, 12, :], 0.0)
        nc.vector.memset(g8[0:1, 12, :], 1.0)
        nc.vector.memset(g8[:, 13, :], 0.0)

        # =========================================================
        # Patch embed (bf16 hi/lo, fp32-quality): hT = pwT @ xpT + b + pos
        # =========================================================
        xph = const.tile([16, N], BF16)
        xpl = const.tile([16, N], BF16)
        pwh = const.tile([16, D], BF16)
        pwl = const.tile([16, D], BF16)
        patch_b = const.tile([128, KT], F32)
        nc.sync.dma_start(out=xph, in_=H["xpT_hi"].ap())
        nc.sync.dma_start(out=xpl, in_=H["xpT_lo"].ap())
        nc.sync.dma_start(out=pwh, in_=H["pw_hi"].ap())
        nc.sync.dma_start(out=pwl, in_=H["pw_lo"].ap())
        nc.sync.dma_start(out=patch_b, in_=H["patch_b"].ap())

        for mt in range(KT):
            pp = ps_s()
            for half in range(NHALF):
                sl = slice(half * 512, half * 512 + 512)
                for i, (w, x) in enumerate(((pwh, xph), (pwh, xpl), (pwl, xph))):
                    nc.tensor.matmul(
                        pp[:, sl], w[:, mt * 128 : mt * 128 + 128], x[:, sl],
                        start=(i == 0), stop=(i == 2),
                    )
            for half in range(NHALF):
                sl = slice(half * 512, half * 512 + 512)
                post = misc.tile([128, 512], F32, tag="res", name="post")
                nc.sync.dma_start(out=post, in_=H["posT"].ap()[:, mt, sl])
                tmp = misc.tile([128, 512], F32, tag="res", name="tmp")
                nc.vector.tensor_scalar_add(
                    out=tmp, in0=pp[:, sl], scalar1=patch_b[:, mt : mt + 1]
                )
                nc.vector.tensor_tensor(
                    out=hT[:, mt, sl], in0=tmp, in1=post, op=OP.add
                )

        # =========================================================
        # LayerNorm -> fp8 slabs (no affine; folded into weights)
        # =========================================================
        def ln_stats(half, mstat):
            """Stats matmuls + mean/var/rstd for one 512-col half.

            mstat: dict receiving mean/rstd tiles keyed by half."""
            sl = slice(half * 512, half * 512 + 512)
            hTb = lnp.tile([128, KT, 512], BF16, tag="hTb", name="hTb")
            sq = lnp.tile([128, KT, 512], BF16, tag="sq", name="sq")
            for k in range(KT):
                nc.vector.tensor_copy(hTb[:, k, :], hT[:, k, sl])
                nc.vector.tensor_tensor(
                    out=sq[:, k, :], in0=hTb[:, k, :], in1=hTb[:, k, :],
                    op=OP.mult,
                )
            pM = ps_mm()
            for k in range(KT):
                nc.tensor.matmul(
                    pM, ones_bf, hTb[:, k, :],
                    start=(k == 0), stop=(k == KT - 1),
                )
            pQ = ps_mm()
            for k in range(KT):
                nc.tensor.matmul(
                    pQ, ones_bf, sq[:, k, :],
                    start=(k == 0), stop=(k == KT - 1),
                )
            mean = stat.tile([128, 512], F32, tag="mean", name="mean")
            nc.vector.tensor_scalar(
                out=mean, in0=pM, scalar1=1.0 / D, scalar2=0.0,
                op0=OP.mult, op1=OP.add,
            )
            # d_ only needs the mean: run it on Pool in parallel with the
            # msq/var/ln/exp rstd chain below.
            ds = []
            for k in range(KT):
                d_ = stat.tile([128, 512], F32, tag="d_", bufs=3, name="d_")
                nc.gpsimd.tensor_tensor(
                    out=d_, in0=hT[:, k, sl], in1=mean, op=OP.subtract
                )
                ds.append(d_)
            msq = stat.tile([128, 512], F32, tag="msq", name="msq")
            nc.vector.tensor_tensor(out=msq, in0=mean, in1=mean, op=OP.mult)
            var = stat.tile([128, 512], F32, tag="var", name="var")
            nc.vector.scalar_tensor_tensor(
                out=var, in0=pQ, scalar=1.0 / D, in1=msq,
                op0=OP.mult, op1=OP.subtract,
            )
            lnv = stat.tile([128, 512], F32, tag="lnv", name="lnv")
            nc.scalar.activation(lnv, var, AF.Ln, bias=eps_col)
            rstd = stat.tile([128, 512], F32, tag="rstd", name="rstd")
            nc.scalar.activation(rstd, lnv, AF.Exp, scale=-0.5)
            mstat[half] = (ds, rstd)

        def ln_chain(dst8, half, mstat):
            """dst8[:, k, half] = (hT - mean)*rstd, fp8."""
            sl = slice(half * 512, half * 512 + 512)
            ds, rstd = mstat[half]
            for k in range(KT):
                nc.gpsimd.tensor_tensor(
                    out=dst8[:, k, sl], in0=ds[k], in1=rstd, op=OP.mult
                )

        # =========================================================
        # fp8 DR projection: out_blocks x halves, contraction via slab pairs
        # =========================================================
        def proj(w_t, src8, nblocks, npairs, evac):
            """psum[b, half] = sum_t w[:, 2t:2t+2, 128b:+128]^T @ src8 pairs."""
            for b_ in range(nblocks):
                pr = [None, None]
                for half in range(NHALF):
                    sl = slice(half * 512, half * 512 + 512)
                    pr[half] = ps_mm()
                    for t in range(npairs):
                        nc.tensor.matmul(
                            pr[half],
                            w_t[:, 2 * t : 2 * t + 2, 128 * b_ : 128 * b_ + 128],
                            src8[:, 2 * t : 2 * t + 2, sl],
                            start=(t == 0), stop=(t == npairs - 1),
                            perf_mode=DR,
                        )
                for half in range(NHALF):
                    evac(pr[half], b_, half)

        # =========================================================
        # Transformer layers
        # =========================================================
        for l in range(n_layers):
            qw = wpool.tile([128, 4, 512], FP8, tag="qw", name=f"qw{l}")
            kw = wpool.tile([128, 4, 512], FP8, tag="kw", name=f"kw{l}")
            vw = wpool.tile([128, 4, D], FP8, tag="vw", name=f"vw{l}")
            ow = wpool.tile([128, 4, D], FP8, tag="ow", name=f"ow{l}")
            mw1 = wpool.tile([128, 4, MLP], FP8, tag="mw1", name=f"mw1{l}")
            mw2 = wpool.tile([128, 14, D], FP8, tag="mw2", name=f"mw2{l}")
            nc.sync.dma_start(out=qw, in_=H["qw8"].ap()[l])
            nc.sync.dma_start(out=kw, in_=H["kw8"].ap()[l])
            nc.sync.dma_start(out=vw, in_=H["vw8"].ap()[l])
            nc.sync.dma_start(out=ow, in_=H["ow8"].ap()[l])
            nc.sync.dma_start(out=mw1, in_=H["mw18"].ap()[l])
            nc.sync.dma_start(out=mw2, in_=H["mw28"].ap()[l])

            # ---- LN1 (h0 was emitted under the previous layer's mlp2) ----
            if l == 0:
                ms1 = {}
                ln_stats(0, ms1)
                ln_chain(xm8, 0, ms1)
            ms1b = {}
            ln_stats(1, ms1b)
            ln_chain(xm8, 1, ms1b)

            # ---- q/k into qdr layout; v into v8p (half-outer) ----
            for half in range(NHALF):
                sl = slice(half * 512, half * 512 + 512)
                for w_t, dst in ((qw, qdr), (kw, kdr)):
                    for b_ in range(4):
                        pr_ = ps_mm()
                        for t in range(2):
                            nc.tensor.matmul(
                                pr_,
                                w_t[:, 2 * t : 2 * t + 2,
                                    128 * b_ : 128 * b_ + 128],
                                xm8[:, 2 * t : 2 * t + 2, sl],
                                start=(t == 0), stop=(t == 1),
                                perf_mode=DR,
                            )
                        nc.vector.tensor_scalar(
                            out=dst[:, b_, sl], in0=pr_, scalar1=WSI,
                            scalar2=0.0, op0=OP.mult, op1=OP.add,
                        )

            def emit_vproj():
                for jt in range(8):
                    jsl = slice(jt * 128, jt * 128 + 128)
                    pv = ps.tile([128, 512], F32, tag="avh", bufs=2, name="ps_v")
                    for t in range(2):
                        nc.tensor.matmul(
                            pv[:, :D],
                            xm8[:, 2 * t : 2 * t + 2, jsl],
                            vw[:, 2 * t : 2 * t + 2, :],
                            start=(t == 0), stop=(t == 1),
                            perf_mode=DR,
                        )
                    vdst = bass.AP(
                        tensor=v8p.tensor,
                        offset=v8p.offset + jt * (2 * D),
                        ap=[list(v8p.ap[0]), [256, KT], [192, 2], [1, 64]],
                    )
                    pv_v = bass.AP(
                        tensor=pv.tensor, offset=pv.offset,
                        ap=[list(pv.ap[0]), [128, KT], [64, 2], [1, 64]],
                    )
                    nc.vector.tensor_scalar(
                        out=vdst, in0=pv_v, scalar1=WSI, scalar2=0.0,
                        op0=OP.mult, op1=OP.add,
                    )

            # ---- attention: pair-pipelined (scores/exp of pair p+1 are
            # emitted before AV/Z of pair p so the ACT exp stream never
            # stalls on the PE's AV/Z tail) ----
            def score_step(hp, par, jt, ex):
                h = 2 * hp + par
                prr = h % 3
                s0 = 2 * (h // 3)
                po = slice(32 * prr, 32 * prr + 32)
                jsl = slice(jt * 128, jt * 128 + 128)
                sp = ps_s()
                for half in range(NHALF):
                    sl = slice(half * 512, half * 512 + 512)
                    nc.tensor.matmul(
                        sp[:, sl],
                        kdr[po, s0 : s0 + 2, jsl],
                        qdr[po, s0 : s0 + 2, sl],
                        start=True, stop=True,
                        perf_mode=DR,
                    )
                nc.scalar.activation(ex[:, jt, :], sp, AF.Exp, scale=1.0 / HD)

            def make_avz(hp, exs):
                state = {}
                steps = []
                for half in range(NHALF):
                    sl = slice(half * 512, half * 512 + 512)
                    for par in range(2):
                        for jp in range(4):
                            def _step(half=half, sl=sl, par=par, jp=jp):
                                if par == 0 and jp == 0:
                                    state["pav"] = ps_avh()
                                    state["pz"] = ps_avh()
                                vsl = slice(256 * hp + 128 * par,
                                            256 * hp + 128 * par + 128)
                                osl = slice(128 * par, 128 * par + 128)
                                st = par == 0 and jp == 0
                                sp_ = par == 1 and jp == 3
                                nc.tensor.matmul(
                                    state["pav"],
                                    v8p[:, 2 * jp : 2 * jp + 2, vsl],
                                    exs[par][:, 2 * jp : 2 * jp + 2, sl],
                                    start=st, stop=sp_, perf_mode=DR,
                                )
                                nc.tensor.matmul(
                                    state["pz"],
                                    ones8z[:, :, osl],
                                    exs[par][:, 2 * jp : 2 * jp + 2, sl],
                                    start=st, stop=sp_, perf_mode=DR,
                                )
                                if sp_:
                                    zrec = expp.tile(
                                        [128, 512], F32, tag="zrec", name="zrec"
                                    )
                                    with nc.allow_low_precision(
                                        reason="1/Z at 18 bits"
                                    ):
                                        nc.vector.reciprocal_approx_fast(
                                            out=zrec, in_=state["pz"]
                                        )
                                    nc.vector.tensor_tensor(
                                        out=oT8[:, hp, sl], in0=state["pav"],
                                        in1=zrec, op=OP.mult,
                                    )
                            steps.append(_step)

                def norm():
                    pass

                return steps, norm

            def vproj_step(jt):
                jsl = slice(jt * 128, jt * 128 + 128)
                pv = ps.tile([128, 512], F32, tag="avh", bufs=2, name="ps_v")
                for t in range(2):
                    nc.tensor.matmul(
                        pv[:, :D],
                        xm8[:, 2 * t : 2 * t + 2, jsl],
                        vw[:, 2 * t : 2 * t + 2, :],
                        start=(t == 0), stop=(t == 1),
                        perf_mode=DR,
                    )
                vdst = bass.AP(
                    tensor=v8p.tensor,
                    offset=v8p.offset + jt * (2 * D),
                    ap=[list(v8p.ap[0]), [256, KT], [192, 2], [1, 64]],
                )
                pv_v = bass.AP(
                    tensor=pv.tensor, offset=pv.offset,
                    ap=[list(pv.ap[0]), [128, KT], [64, 2], [1, 64]],
                )
                nc.vector.tensor_scalar(
                    out=vdst, in0=pv_v, scalar1=WSI, scalar2=0.0,
                    op0=OP.mult, op1=OP.add,
                )

            # scores/exp of pair p interleaved, step-by-step, with AV/Z of
            # pair p-1 (and with v-proj during pair 0) so the PE never
            # drains while the ACT exp stream runs.
            avz_steps = avz_norm = None
            for hp in range(NH // 2):
                exs = [
                    expp.tile([128, 8, N], FP8, tag="ex", bufs=4, name=f"ex{2*hp+par}")
                    for par in range(2)
                ]
                for i in range(16):
                    par, jt = divmod(i, 8)
                    score_step(hp, par, jt, exs[par])
                    if avz_steps is not None:
                        avz_steps[i]()
                    elif i % 2 == 1:
                        vproj_step(i // 2)
                if avz_norm is not None:
                    avz_norm()
                avz_steps, avz_norm = make_avz(hp, exs)
            for st in avz_steps:
                st()
            avz_norm()

            # ---- o-proj + residual (half-outer) ----
            def proj_half(w_t, src8, nblocks, npairs, half, evac):
                sl = slice(half * 512, half * 512 + 512)
                for b_ in range(nblocks):
                    pr_ = ps_mm()
                    for t in range(npairs):
                        nc.tensor.matmul(
                            pr_,
                            w_t[:, 2 * t : 2 * t + 2,
                                128 * b_ : 128 * b_ + 128],
                            src8[:, 2 * t : 2 * t + 2, sl],
                            start=(t == 0), stop=(t == npairs - 1),
                            perf_mode=DR,
                        )
                    evac(pr_, b_, sl)

            def evac_resid(pr_, b_, sl):
                nc.vector.scalar_tensor_tensor(
                    out=hT[:, b_, sl], in0=pr_, scalar=WSI,
                    in1=hT[:, b_, sl], op0=OP.mult, op1=OP.add,
                )

            proj_half(ow, oT8, 3, 2, 0, evac_resid)
            proj_half(ow, oT8, 3, 2, 1, evac_resid)
            ms2 = {}
            ln_stats(0, ms2)
            ln_chain(xm28, 0, ms2)
            ms2b = {}
            ln_stats(1, ms2b)
            ln_chain(xm28, 1, ms2b)

            # mlp1: pair two 128-row blocks per [128, 1024] psum; one gelu
            # per pair writes both g8 slabs (strided out AP)
            for half in range(NHALF):
                sl = slice(half * 512, half * 512 + 512)
                for bp in range(MLP // 256):
                    pg = ps_mm_big()
                    for sub in range(2):
                        b_ = 2 * bp + sub
                        psl = slice(sub * 512, sub * 512 + 512)
                        for t in range(2):
                            nc.tensor.matmul(
                                pg[:, psl],
                                mw1[:, 2 * t : 2 * t + 2,
                                    128 * b_ : 128 * b_ + 128],
                                xm28[:, 2 * t : 2 * t + 2, sl],
                                start=(t == 0), stop=(t == 1),
                                perf_mode=DR,
                            )
                    gdst = bass.AP(
                        tensor=g8.tensor,
                        offset=g8.offset + 2 * bp * N + half * 512,
                        ap=[list(g8.ap[0]), [N, 2], [1, 512]],
                    )
                    nc.scalar.activation(gdst, pg, AF.Gelu, scale=WSI)

            # mlp2 (half-outer so next layer's LN1(h0) can start early)
            proj_half(mw2, g8, 3, 7, 0, evac_resid)
            if l + 1 < n_layers:
                # next layer's LN1 half0 hides under mlp2 half1
                ms1n = {}
                ln_stats(0, ms1n)
                proj_half(mw2, g8, 3, 7, 1, evac_resid)
                ln_chain(xm8, 0, ms1n)
            else:
                proj_half(mw2, g8, 3, 7, 1, evac_resid)

        # =========================================================
        # Final layer: plain LN (affine folded into fpw) in bf16 hi/lo
        # =========================================================
        fpwh = const.tile([128, KT, 16], BF16)
        fpwl = const.tile([128, KT, 16], BF16)
        fpb = const.tile([16, 1], F32)
        nc.sync.dma_start(out=fpwh, in_=H["fpw_hi"].ap())
        nc.sync.dma_start(out=fpwl, in_=H["fpw_lo"].ap())
        nc.sync.dma_start(out=fpb, in_=H["fpb"].ap())

        xmF = lnp.tile([128, KT, N], BF16, tag="hTb", name="xmF")
        xmFlo = lnp.tile([128, KT, N], BF16, tag="sq", name="xmFlo")
        hTb = lnp.tile([128, KT, N], BF16, tag="hTb", name="hTbF")
        sq = lnp.tile([128, KT, N], BF16, tag="sq", name="sqF")
        for k in range(KT):
            nc.vector.tensor_copy(hTb[:, k, :], hT[:, k, :])
            nc.vector.tensor_tensor(
                out=sq[:, k, :], in0=hTb[:, k, :], in1=hTb[:, k, :], op=OP.mult
            )
        for half in range(NHALF):
            sl = slice(half * 512, half * 512 + 512)
            pM = ps_mm()
            for k in range(KT):
                nc.tensor.matmul(
                    pM, ones_bf, hTb[:, k, sl],
                    start=(k == 0), stop=(k == KT - 1),
                )
            pQ = ps_mm()
            for k in range(KT):
                nc.tensor.matmul(
                    pQ, ones_bf, sq[:, k, sl],
                    start=(k == 0), stop=(k == KT - 1),
                )
            mean = stat.tile([128, 512], F32, tag="mean", name="meanF")
            nc.vector.tensor_scalar(
                out=mean, in0=pM, scalar1=1.0 / D, scalar2=0.0,
                op0=OP.mult, op1=OP.add,
            )
            msq = stat.tile([128, 512], F32, tag="msq", name="msqF")
            nc.vector.tensor_tensor(out=msq, in0=mean, in1=mean, op=OP.mult)
            var = stat.tile([128, 512], F32, tag="var", name="varF")
            nc.vector.scalar_tensor_tensor(
                out=var, in0=pQ, scalar=1.0 / D, in1=msq,
                op0=OP.mult, op1=OP.subtract,
            )
            lnv = stat.tile([128, 512], F32, tag="lnv", name="lnvF")
            nc.scalar.activation(lnv, var, AF.Ln, bias=eps_col)
            rstd = stat.tile([128, 512], F32, tag="rstd", name="rstdF")
            nc.scalar.activation(rstd, lnv, AF.Exp, scale=-0.5)
            for k in range(KT):
                d_ = stat.tile([128, 512], F32, tag="d_", bufs=3, name="dF")
                nc.vector.scalar_tensor_tensor(
                    out=d_, in0=mean, scalar=-1.0, in1=hT[:, k, sl],
                    op0=OP.mult, op1=OP.add,
                )
                ef = stat.tile([128, 512], F32, tag="ef", name="ef")
                nc.vector.tensor_tensor(out=ef, in0=d_, in1=rstd, op=OP.mult)
                nc.vector.tensor_copy(xmF[:, k, sl], ef)
                nc.vector.scalar_tensor_tensor(
                    out=xmFlo[:, k, sl], in0=xmF[:, k, sl], scalar=-1.0,
                    in1=ef, op0=OP.mult, op1=OP.add,
                )

        out_sb = misc.tile([16, N], F32, tag="outsb", bufs=1, name="out_sb")
        for half in range(NHALF):
            sl = slice(half * 512, half * 512 + 512)
            pf = ps_mm()
            mms = []
            for k in range(KT):
                mms.append((fpwh[:, k, :], xmF[:, k, sl]))
                mms.append((fpwl[:, k, :], xmF[:, k, sl]))
                mms.append((fpwh[:, k, :], xmFlo[:, k, sl]))
            for i, (wv, xv) in enumerate(mms):
                nc.tensor.matmul(
                    pf[:16, :], wv, xv,
                    start=(i == 0), stop=(i == len(mms) - 1),
                )
            nc.vector.tensor_scalar_add(
                out=out_sb[:, sl], in0=pf[:16, :], scalar1=fpb
            )
        nc.sync.dma_start(out=H["outT"].ap(), in_=out_sb)


# =================================================================
# Host side
# =================================================================
_BUILD_CACHE = {}


def _get_module(n_layers=L):
    if n_layers not in _BUILD_CACHE:
        _register_profile_hook()
        _BUILD_CACHE[n_layers] = build_module(n_layers)
    return _BUILD_CACHE[n_layers]


def _silu(x):
    return x / (1.0 + np.exp(-x))


def _t_embed_np(t, w1, b1, w2, b2):
    half = FREQ // 2
    freqs = np.exp(-math.log(10000.0) * np.arange(half, dtype=np.float64) / half)
    a = t.astype(np.float64)[:, None] * freqs[None]
    e = np.concatenate([np.cos(a), np.sin(a)], -1)
    return _silu(e @ w1 + b1) @ w2 + b2


def _fp8(x):
    return np.clip(np.asarray(x, np.float32), -440.0, 440.0).astype(E4M3)


def _shuf_w(w):
    """[Din, Dout] -> [128, Din//128, Dout], partition-contiguous."""
    din = w.shape[0]
    return np.ascontiguousarray(w.reshape(din // 128, 128, -1).transpose(1, 0, 2))


def _dr_w(w_f, bias, dout_pad=None):
    """[D, Dout] + bias[Dout] -> [128, 4, Dout] fp8 with pad slab bias row."""
    dout = w_f.shape[1] if dout_pad is None else dout_pad
    out = np.zeros((128, 4, dout), np.float32)
    s = _shuf_w(w_f * WS)  # [128, 3, Dout]
    out[:, 0:3, : w_f.shape[1]] = s
    out[0, 3, : w_f.shape[1]] = bias * WS
    return _fp8(out)


def prepare_inputs(inputs, n_layers=L):
    ii = {k: np.asarray(v) for k, v in inputs.items()}
    x = ii["x"].astype(np.float32)
    t = ii["t"].astype(np.float64)
    dt = ii["dt"].astype(np.float64)
    y = ii["y"].astype(np.int64)
    pos = ii["pos"].astype(np.float32)

    # ---- conditioning on host ----
    c = (
        _t_embed_np(t, ii["t1_w1"].astype(np.float64), ii["t1_b1"].astype(np.float64),
                    ii["t1_w2"].astype(np.float64), ii["t1_b2"].astype(np.float64))
        + _t_embed_np(dt, ii["t2_w1"].astype(np.float64), ii["t2_b1"].astype(np.float64),
                      ii["t2_w2"].astype(np.float64), ii["t2_b2"].astype(np.float64))
        + ii["label_emb"].astype(np.float64)[y]
    )  # [B, D]
    ca = _silu(c)  # [B, D]

    shared = {}
    shared["posT"] = _shuf_w(np.ascontiguousarray(pos.T))
    pw = ii["patch_w"].astype(np.float32)
    pwh = pw.astype(ml_dtypes.bfloat16)
    shared["pw_hi"] = pwh
    shared["pw_lo"] = (pw - pwh.astype(np.float32)).astype(ml_dtypes.bfloat16)
    shared["patch_b"] = np.ascontiguousarray(
        ii["patch_b"].astype(np.float32).reshape(-1, 128).T
    )

    in_maps = []
    for b in range(B):
        m = dict(shared)
        xp = (
            x[b]
            .reshape(CIN, HP, P, HP, P)
            .transpose(1, 3, 0, 2, 4)
            .reshape(N, CIN * P * P)
        )
        xpT = np.ascontiguousarray(xp.T)
        xph = xpT.astype(ml_dtypes.bfloat16)
        m["xpT_hi"] = xph
        m["xpT_lo"] = (xpT - xph.astype(np.float32)).astype(ml_dtypes.bfloat16)

        qw8 = np.zeros((L, 128, 4, 512), E4M3)
        kw8 = np.zeros((L, 128, 4, 512), E4M3)
        vw8 = np.zeros((L, 128, 4, D), E4M3)
        ow8 = np.zeros((L, 128, 4, D), E4M3)
        mw18 = np.zeros((L, 128, 4, MLP), E4M3)
        mw28 = np.zeros((L, 128, 14, D), E4M3)

        for l in range(n_layers):
            mod = ca[b] @ ii["adaln_w"][l].astype(np.float64) + ii["adaln_b"][
                l
            ].astype(np.float64)
            sh_a, sc_a, g_a, sh_m, sc_m, g_m = np.split(mod, 6)
            ga = (1.0 + sc_a).astype(np.float32)
            ba = sh_a.astype(np.float32)
            gm = (1.0 + sc_m).astype(np.float32)
            bm = sh_m.astype(np.float32)
            g_a = g_a.astype(np.float32)
            g_m = g_m.astype(np.float32)

            # q/k: fold LN affine; permute out cols into qdr block layout
            for nm, dst in (("q", qw8), ("k", kw8)):
                w_f = ga[:, None] * ii[f"{nm}_w"][l].astype(np.float32)
                b_f = ii[f"{nm}_b"][l].astype(np.float32) + ba @ ii[f"{nm}_w"][
                    l
                ].astype(np.float32)
                wp = np.zeros((D, 512), np.float32)
                bp = np.zeros((512,), np.float32)
                for blk in range(4):
                    cols = QK_PERM[blk][QK_USED[blk]]
                    idx = np.nonzero(QK_USED[blk])[0]
                    wp[:, 128 * blk + idx] = w_f[:, cols]
                    bp[128 * blk + idx] = b_f[cols]
                dst[l] = _dr_w(wp, bp)

            # v: fold LN affine; bias folded into o bias
            vw_f = ga[:, None] * ii["v_w"][l].astype(np.float32)
            vb_eff = ii["v_b"][l].astype(np.float32) + ba @ ii["v_w"][l].astype(
                np.float32
            )
            vw8[l] = _dr_w(vw_f, np.zeros(D, np.float32))

            # o: fold gate; bias absorbs v bias
            ow_f = ii["o_w"][l].astype(np.float32) * g_a[None, :]
            ob_f = g_a * (
                ii["o_b"][l].astype(np.float32)
                + vb_eff @ ii["o_w"][l].astype(np.float32)
            )
            ow8[l] = _dr_w(ow_f, ob_f)

            # mlp1: fold LN affine
            mw1_f = gm[:, None] * ii["m_w1"][l].astype(np.float32)
            mb1_f = ii["m_b1"][l].astype(np.float32) + bm @ ii["m_w1"][l].astype(
                np.float32
            )
            mw18[l] = _dr_w(mw1_f, mb1_f)

            # mlp2: fold gate; 14-slab layout with bias row in slab 12
            mw2_f = ii["m_w2"][l].astype(np.float32) * g_m[None, :]
            mb2_f = g_m * ii["m_b2"][l].astype(np.float32)
            w14 = np.zeros((128, 14, D), np.float32)
            w14[:, 0:12, :] = _shuf_w(mw2_f * WS)
            w14[0, 12, :] = mb2_f * WS
            mw28[l] = _fp8(w14)

        m["qw8"] = qw8
        m["kw8"] = kw8
        m["vw8"] = vw8
        m["ow8"] = ow8
        m["mw18"] = mw18
        m["mw28"] = mw28

        # final layer fold
        finm = _silu(c[b]) @ ii["fin_mw"].astype(np.float64) + ii["fin_mb"].astype(
            np.float64
        )
        sh_f, sc_f = np.split(finm, 2)
        gf = (1.0 + sc_f).astype(np.float32)
        bf = sh_f.astype(np.float32)
        fpw_f = gf[:, None] * ii["fin_pw"].astype(np.float32)
        fpb_f = ii["fin_pb"].astype(np.float32) + bf @ ii["fin_pw"].astype(
            np.float32
        )
        fpw_s = _shuf_w(fpw_f)
        fpwh = fpw_s.astype(ml_dtypes.bfloat16)
        m["fpw_hi"] = fpwh
        m["fpw_lo"] = (fpw_s - fpwh.astype(np.float32)).astype(ml_dtypes.bfloat16)
        m["fpb"] = fpb_f.reshape(16, 1)
        in_maps.append(m)
    return in_maps


def assemble_output(results):
    out = np.empty((B, COUT, IMG, IMG), np.float32)
    for b in range(B):
        tok = results[b]["outT"].T  # [N, 16]
        out[b] = (
            tok.reshape(HP, HP, P, P, COUT)
            .transpose(4, 0, 2, 1, 3)
            .reshape(COUT, IMG, IMG)
        )
    return out


def run(inputs, n_layers=L, trace=False, sim=False):
    nc = _get_module(n_layers)
    in_maps = prepare_inputs(inputs, n_layers)
    if sim:
        from concourse.bass_interp import CoreSim

        s = CoreSim(nc, trace=False)
        for k, v in in_maps[0].items():
            s.tensor(k)[:] = v
        s.simulate()
        results = [{"outT": np.array(s.tensor("outT"))} for _ in range(B)]
        return results, None
    res = run_bass_kernel_spmd(
        nc, in_maps, core_ids=list(range(B)), trace=trace
    )
    return res.results, res


def kernel(**inputs):
    results, _ = run(inputs, L, trace=False, sim=False)
    return assemble_output(results)
